# revision 1
# baseline (speedup 1.0000x reference)
"""nn_ContourIntegrationLayer — depthwise 3x3 lateral conv (zero center) + residual.

Strategy: data-parallel over batch (4 images/core on 8 cores). Host transposes
to channels-first and pads spatially; the device kernel then works in a
channels-on-partitions layout with no on-device transposes:

  - per core: 8 "planes" [128ch, 58, 58] f32 (4 batches x 2 channel halves)
  - center tap is zeroed by the reference and a residual add follows, so the
    effective kernel has center coefficient 1.0 -> one 9-tap depthwise conv
  - 6 planes on the TensorEngine: 9 accumulating diagonal matmuls per 8-row
    chunk (float32r moving/stationary operands: 1 cycle/row), PSUM f32
    accumulation, ScalarE drains PSUM -> SBUF
  - 2 planes on the VectorEngine: tensor_scalar + 7 scalar_tensor_tensor
    per-partition-scalar MACs + a tensor_tensor center add, all fp32
  - all DMAs are plain f32 HWDGE transfers (big contiguous per-partition runs)

Host gathers the 8 per-core outputs and transposes back to channels-last.
"""

import numpy as np

F32 = None  # set lazily in _build (mybir import kept inside functions)

_R, _C, _CH = 56, 56, 256
_RP, _CP = 58, 58
_CHUNK = 8
_NCHUNKS = _R // _CHUNK
_NCORES = 8
_TAPS = [(dr, dc) for dr in (-1, 0, 1) for dc in (-1, 0, 1)]  # index 4 = center

_CACHE = {}


def _host_prep(x, kern):
    B = x.shape[0]
    xt = np.ascontiguousarray(np.transpose(x, (0, 3, 1, 2)))  # [B, CH, R, C]
    xp = np.zeros((B, _CH, _RP, _CP), np.float32)
    xp[:, :, 1:1 + _R, 1:1 + _C] = xt
    k = np.array(kern, np.float32).copy()
    k[1, 1, :] = 1.0  # ZeroCenter constraint + residual add => center coeff 1.0
    kvec = np.zeros((2, 9, 128), np.float32)
    kdiag = np.zeros((2, 9, 128, 128), np.float32)
    for h in range(2):
        for t, (dr, dc) in enumerate(_TAPS):
            kvec[h, t] = k[dr + 1, dc + 1, 128 * h:128 * (h + 1)]
            np.fill_diagonal(kdiag[h, t], kvec[h, t])
    bsh = B // _NCORES
    shards = [np.ascontiguousarray(xp[i * bsh:(i + 1) * bsh])
              for i in range(_NCORES)]
    return shards, kvec, kdiag, bsh


def _build(bsh):
    import concourse.bacc as bacc
    import concourse.mybir as mybir
    import concourse.tile as tile

    F32 = mybir.dt.float32
    F32R = mybir.dt.float32r

    planes = [(b, h) for b in range(bsh) for h in range(2)]
    n = len(planes)
    dve_planes = {2, 3} if n >= 6 else set()

    nc = bacc.Bacc()
    x_d = nc.declare_dram_parameter("xt", [bsh, _CH, _RP, _CP], F32R, isOutput=False)
    kv_d = nc.declare_dram_parameter("kvec", [2, 9, 128], F32, isOutput=False)
    kd_d = nc.declare_dram_parameter("kdiag", [2, 9, 128, 128], F32R, isOutput=False)
    out_d = nc.declare_dram_parameter("out", [bsh, _CH, _R, _C], F32, isOutput=True)

    with tile.TileContext(nc) as tc:
        with tc.tile_pool(name="const", bufs=1) as cpool, \
             tc.tile_pool(name="xin", bufs=1) as xpool, \
             tc.tile_pool(name="oout", bufs=5) as opool, \
             tc.tile_pool(name="ps", bufs=4, space="PSUM") as ppool:

            kv, kd = [], []
            for h in range(2):
                kvh = cpool.tile([128, 9], F32, name=f"kv{h}")
                nc.sync.dma_start(out=kvh[:], in_=kv_d.ap()[h].rearrange("t p -> p t"))
                kv.append(kvh)
                kdh = cpool.tile([128, 9, 128], F32R, name=f"kd{h}")
                nc.sync.dma_start(out=kdh[:], in_=kd_d.ap()[h].rearrange("t a b -> a t b"))
                kd.append(kdh)

            xts = []
            for i, (b, h) in enumerate(planes):
                xt_i = xpool.tile([128, _RP, _CP], F32R, name=f"x{i}")
                nc.sync.dma_start(out=xt_i[:], in_=x_d.ap()[b, 128 * h:128 * (h + 1)])
                xts.append(xt_i)

            for i, (b, h) in enumerate(planes):
                xt_i = xts[i]
                ot = opool.tile([128, _R, _C], F32, name="ot", tag="ot")
                if i not in dve_planes:
                    for ci in range(_NCHUNKS):
                        r0 = 1 + _CHUNK * ci
                        ps = ppool.tile([128, _CHUNK, _C], F32, name="ps")
                        for t, (dr, dc) in enumerate(_TAPS):
                            nc.tensor.matmul(
                                ps[:], kd[h][:, t, :],
                                xt_i[:, r0 + dr:r0 + dr + _CHUNK,
                                     1 + dc:1 + dc + _C],
                                start=(t == 0), stop=(t == 8))
                        nc.scalar.copy(
                            ot[:, _CHUNK * ci:_CHUNK * (ci + 1), :], ps[:])
                else:
                    def src(dr, dc):
                        return xt_i[:, 1 + dr:1 + dr + _R, 1 + dc:1 + dc + _C].bitcast(F32)
                    lat = [(t, dr, dc) for t, (dr, dc) in enumerate(_TAPS)
                           if (dr, dc) != (0, 0)]
                    t0, dr0, dc0 = lat[0]
                    nc.vector.tensor_scalar_mul(
                        ot[:], src(dr0, dc0), kv[h][:, t0:t0 + 1])
                    for t, dr, dc in lat[1:]:
                        nc.vector.scalar_tensor_tensor(
                            out=ot[:], in0=src(dr, dc), scalar=kv[h][:, t:t + 1],
                            in1=ot[:], op0=mybir.AluOpType.mult,
                            op1=mybir.AluOpType.add)
                    nc.vector.tensor_add(ot[:], src(0, 0), ot[:])
                nc.sync.dma_start(out=out_d.ap()[b, 128 * h:128 * (h + 1)],
                                  in_=ot[:])
    nc.finalize()
    return nc


def run(x, kern, trace=False):
    """Returns (out [B,56,56,256] f32, exec_time_ns or None)."""
    from concourse.bass_utils import run_bass_kernel_spmd

    x = np.asarray(x, np.float32)
    kern = np.asarray(kern, np.float32)
    shards, kvec, kdiag, bsh = _host_prep(x, kern)

    if "nc" not in _CACHE or _CACHE.get("bsh") != bsh:
        _CACHE["nc"] = _build(bsh)
        _CACHE["bsh"] = bsh
    nc = _CACHE["nc"]

    in_maps = [{"xt": shards[i], "kvec": kvec, "kdiag": kdiag}
               for i in range(_NCORES)]
    try:
        res = run_bass_kernel_spmd(nc, in_maps, core_ids=list(range(_NCORES)),
                                   trace=trace)
    except ModuleNotFoundError:
        # NTFF profile hook unavailable in this container; run untraced.
        res = run_bass_kernel_spmd(nc, in_maps, core_ids=list(range(_NCORES)),
                                   trace=False)
    outs = [np.asarray(res.results[i]["out"]) for i in range(_NCORES)]
    out = np.concatenate(outs, axis=0)            # [B, CH, R, C]
    out = np.ascontiguousarray(np.transpose(out, (0, 2, 3, 1)))
    return out.astype(np.float32), res.exec_time_ns


def kernel(x, kernel):
    out, _ = run(x, kernel, trace=False)
    return out

